# revision 22
# baseline (speedup 1.0000x reference)
"""Bass/Trainium2 kernel for memory-augmented causal self-attention.

Sharding (hardcoded): 8 cores = 2 batches x 4 head-groups (3 heads each).
Each core computes its batch's attention for its 3 heads plus the
row-sharded output projection partial; the host sums the 4 partials.

Self-contained: no reads of reference.py/spec.json.
"""
import os
import sys

for _p in ("/opt/trn_rl_repo",):
    if _p not in sys.path:
        sys.path.append(_p)

import numpy as np
import concourse.bass as bass
import concourse.tile as tile
from concourse import bacc, mybir
from concourse.masks import make_identity

F32 = mybir.dt.float32
F32R = mybir.dt.float32r
AF = mybir.ActivationFunctionType
OP = mybir.AluOpType

# Problem shapes (hardcoded from the task spec)
B, T, C = 2, 2048, 768
H, HD = 12, 64
MT = 64                  # memory tokens
HPC = 3                  # heads per core
S = MT + T               # 2112 keys
NJ = T // 512            # 4 q-tiles of 512
NT = T // 128            # 16 T-chunks of 128
KC = C // 128            # 6 contraction chunks

# qkv projection M-chunk layout (5 chunks of 128 output channels):
#   chunk0 = [q0|q1]  chunk1 = [k0|k1]  chunk2 = [q2|k2]
#   chunk3 = [v0|v1]  chunk4 = [v2|q2dup]
NEG = -1e30


def _nchunks(j):
    # S-chunks for q-tile j: chunk 0 = mem (64 rows), chunks 1..4(j+1) = v blocks
    return 4 * (j + 1) + 1


DEBUG = bool(int(os.environ.get("KERNEL_DEBUG", "0")))


def build_nc():
    nc = bacc.Bacc()
    x_d = nc.declare_dram_parameter("x", [T, C], F32, isOutput=False)
    mem_d = nc.declare_dram_parameter("mem", [MT, C], F32, isOutput=False)
    wqkv_d = nc.declare_dram_parameter("wqkv", [C, 640], F32, isOutput=False)
    bqkv_d = nc.declare_dram_parameter("bqkv", [1, 640], F32, isOutput=False)
    wmem_d = nc.declare_dram_parameter("wmem", [C, 448], F32, isOutput=False)
    bmemk_d = nc.declare_dram_parameter("bmemk", [1, 256], F32, isOutput=False)
    bmemv_d = nc.declare_dram_parameter("bmemv", [1, 192], F32, isOutput=False)
    wproj_d = nc.declare_dram_parameter("wproj", [193, C], F32, isOutput=False)
    out_d = nc.declare_dram_parameter("out", [T, C], F32, isOutput=True)
    dbg = {}
    if DEBUG:
        for name, shape in (("d_xT", [128, KC * T]), ("d_qT01", [128, T]),
                            ("d_fkT01", [128, S]), ("d_q2k2", [128, S]),
                            ("d_v01T", [128, T]), ("d_v2q2", [128, T]),
                            ("d_fv0", [128, 17 * 65]), ("d_fv2", [128, 17 * 65]),
                            ("d_yA", [128, T]), ("d_yB", [128, T]),
                            ("d_m01", [128, 512]), ("d_memT", [128, KC * MT]),
                            ("d_acc0", [128, 512]), ("d_den0", [1, 512]),
                            ("d_bc0", [64, 512]), ("d_et0", [128, 512])):
            dbg[name] = nc.declare_dram_parameter(name, shape, F32, isOutput=True)

    with tile.TileContext(nc) as tc:
        import contextlib
        with contextlib.ExitStack() as ctx:
            _build_tile(ctx, tc, nc, x_d, mem_d, wqkv_d, bqkv_d, wmem_d,
                        bmemk_d, bmemv_d, wproj_d, out_d, dbg)
    nc.finalize()
    return nc


def _build_tile(ctx, tc, nc, x_d, mem_d, wqkv_d, bqkv_d, wmem_d,
                bmemk_d, bmemv_d, wproj_d, out_d, dbg=None):
    # ---- pools ----
    big = ctx.enter_context(tc.tile_pool(name="big", bufs=1))       # persistents
    stage = ctx.enter_context(tc.tile_pool(name="stage", bufs=2))   # x staging
    expp = ctx.enter_context(tc.tile_pool(name="expp", bufs=4))     # exp tiles
    smalls = ctx.enter_context(tc.tile_pool(name="smalls", bufs=2)) # denom/recip
    outst = ctx.enter_context(tc.tile_pool(name="outst", bufs=3))   # out staging
    psA = ctx.enter_context(tc.tile_pool(name="psA", bufs=3, space="PSUM"))
    psX = ctx.enter_context(tc.tile_pool(name="psX", bufs=3, space="PSUM"))
    psV = ctx.enter_context(tc.tile_pool(name="psV", bufs=2, space="PSUM"))

    # ---- persistent SBUF ----
    xT = big.tile([128, KC, T], F32R, tag="xT")          # x^T, chunk c = [:, c, :]
    memT = big.tile([128, KC, MT], F32R, tag="memT")
    wqkv = big.tile([128, KC, 640], F32R, tag="wqkv")
    wmem = big.tile([128, KC, 448], F32R, tag="wmem")
    wproj = big.tile([128, 2, C], F32R, tag="wproj")
    bqkv = big.tile([128, 5], F32, tag="bqkv")
    bmemk = big.tile([128, 2], F32, tag="bmemk")
    bmemv = big.tile([1, 192], F32R, tag="bmemv")
    qT01 = big.tile([128, T], F32R, tag="qT01")          # rows 0:64 q_h0, 64:128 q_h1
    fkT01 = big.tile([128, S], F32R, tag="fkT01")        # [mk|k] same row split
    q2k2 = big.tile([128, S], F32R, tag="q2k2")          # rows 0:64 = qT2 (cols 64:S); rows 64:128 = fkT2
    v01T = big.tile([128, T], F32R, tag="v01T")
    v2q2 = big.tile([128, T], F32R, tag="v2q2")          # rows 0:64 vT2, 64:128 qT2 dup
    # fv per head: [128, chunk, 65]; col 64 = ones (denominator trick)
    fv = [big.tile([128, 17, 65], F32R, tag=f"fv{h}", name=f"fv{h}") for h in range(HPC)]
    yA = big.tile([128, T], F32R, tag="yA")              # rows 0:64 h0, 64:128 h1 (normalized y^T)
    yB = big.tile([128, T], F32R, tag="yB")              # rows 0:64 h2, row 64 = ones (bias row)
    ident = big.tile([128, 128], F32R, tag="ident")
    m01 = big.tile([128, 512], F32R, tag="m01")          # causal multiplier m01[p,i] = p<=i
    ones1 = big.tile([1, 64], F32R, tag="ones1")

    # ---- constants ----
    ident_stg = stage.tile([128, 128], F32, tag="identstg")
    make_identity(nc, ident_stg)
    nc.vector.tensor_copy(ident, ident_stg)
    # m01[p, i] = (p - i) <= 0 ? in_(1.0) : fill(0.0)
    nc.gpsimd.memset(m01.bitcast(F32), 1.0)
    # expr = i - p >= 0  <=>  p <= i
    nc.gpsimd.affine_select(out=m01.bitcast(F32), in_=m01.bitcast(F32),
                            compare_op=OP.is_ge, fill=0.0, base=0,
                            pattern=[[1, 512]], channel_multiplier=-1)
    nc.gpsimd.memset(ones1.bitcast(F32), 1.0)
    for h in range(HPC):
        nc.gpsimd.memset(fv[h][:, :, 64:65].bitcast(F32), 1.0)
    nc.gpsimd.memset(yB[64:65, :].bitcast(F32), 1.0)

    # ---- weight / bias DMAs ----
    nc.sync.dma_start(out=wqkv, in_=wqkv_d[:, :].bitcast(F32R).rearrange("(c p) m -> p c m", p=128))
    nc.sync.dma_start(out=wmem, in_=wmem_d[:, :].bitcast(F32R).rearrange("(c p) m -> p c m", p=128))
    nc.sync.dma_start(out=wproj[:, 0, :], in_=wproj_d[0:128, :].bitcast(F32R))
    nc.sync.dma_start(out=wproj[0:65, 1, :], in_=wproj_d[128:193, :].bitcast(F32R))
    nc.sync.dma_start(out=bqkv, in_=bqkv_d[0, :].rearrange("(m p) -> p m", p=128))
    nc.sync.dma_start(out=bmemk, in_=bmemk_d[0, :].rearrange("(m p) -> p m", p=128))
    nc.sync.dma_start(out=bmemv, in_=bmemv_d[:, :].bitcast(F32R))

    # ---- mem: load, transpose, project ----
    mstg = stage.tile([MT, C], F32R, tag="mstg")
    nc.sync.dma_start(out=mstg, in_=mem_d[:, :].bitcast(F32R))
    for c in range(KC):
        pt = psX.tile([128, 512], F32R, tag="ps")
        nc.tensor.transpose(pt[:, 0:MT], mstg[:, 128 * c:128 * (c + 1)], ident[0:MT, 0:MT])
        nc.vector.tensor_copy(memT[:, c, :], pt[:, 0:MT])
    # mk^T chunks -> fkT cols 0:MT.  chunk0=[mk0|mk1] -> fkT01; chunk1=[pad|mk2] -> q2k2 rows 64:128
    for mchunk, (dst, rows) in enumerate(((fkT01, slice(0, 128)), (q2k2, slice(64, 128)))):
        pt = psX.tile([128, 512], F32, tag="ps")
        for c in range(KC):
            nc.tensor.matmul(pt[:, 0:MT], wmem[:, c, 128 * mchunk:128 * (mchunk + 1)],
                             memT[:, c, :], start=(c == 0), stop=(c == KC - 1))
        nc.vector.tensor_scalar(out=dst[rows, 0:MT], in0=pt[rows, 0:MT],
                                scalar1=bmemk[rows, mchunk:mchunk + 1], scalar2=None,
                                op0=OP.add)
    # mv natural [tokens, dims] for fv chunk 0 (+ bias via K=1 ones row)
    pt = psX.tile([128, 512], F32, tag="ps")
    for c in range(KC):
        nc.tensor.matmul(pt[0:MT, 0:192], memT[:, c, :], wmem[:, c, 256:448],
                         start=(c == 0), stop=False)
    nc.tensor.matmul(pt[0:MT, 0:192], ones1, bmemv, start=False, stop=True)
    for h in range(HPC):
        nc.vector.tensor_copy(fv[h][0:MT, 0, 0:64], pt[0:MT, 64 * h:64 * (h + 1)])

    # ---- x: load + transpose ----
    for t in range(NT):
        xs = stage.tile([128, C], F32R, tag="xs")
        nc.sync.dma_start(out=xs, in_=x_d[128 * t:128 * (t + 1), :].bitcast(F32R))
        for c in range(KC):
            pt = psX.tile([128, 512], F32R, tag="ps")
            nc.tensor.transpose(pt[:, 0:128], xs[:, 128 * c:128 * (c + 1)], ident)
            nc.vector.tensor_copy(xT[:, c, 128 * t:128 * (t + 1)], pt[:, 0:128])

    # ---- qkv projection + attention + output projection, per q-tile j ----
    # proj M-chunk destinations: (tensor, col offset in that tensor)
    def proj_dst(m):
        return ((qT01, 0), (fkT01, MT), (q2k2, MT), (v01T, 0), (v2q2, 0))[m]

    for j in range(NJ):
        q0, q1 = 512 * j, 512 * (j + 1)
        # --- projection for this q-tile: 5 M-chunks x 6 K matmuls ---
        for m in range(5):
            pt = psA.tile([128, 512], F32, tag="pa")
            for c in range(KC):
                nc.tensor.matmul(pt, wqkv[:, c, 128 * m:128 * (m + 1)],
                                 xT[:, c, q0:q1], start=(c == 0), stop=(c == KC - 1))
            dst, off = proj_dst(m)
            nc.vector.tensor_scalar(out=dst[:, off + q0:off + q1], in0=pt,
                                    scalar1=bqkv[:, m:m + 1], scalar2=None, op0=OP.add)
        # --- v transposes for blocks 4j..4j+3 -> fv chunks 4j+1..4j+4 ---
        for b in range(4 * j, 4 * (j + 1)):
            cidx = b + 1
            pt = psX.tile([128, 512], F32R, tag="ps")
            nc.tensor.transpose(pt[:, 0:128], v01T[:, 128 * b:128 * (b + 1)], ident)
            nc.vector.tensor_copy(fv[0][:, cidx, 0:64], pt[:, 0:64])
            nc.vector.tensor_copy(fv[1][:, cidx, 0:64], pt[:, 64:128])
            pt2 = psX.tile([128, 512], F32R, tag="ps")
            nc.tensor.transpose(pt2[:, 0:64], v2q2[0:64, 128 * b:128 * (b + 1)], ident[0:64, 0:64])
            nc.vector.tensor_copy(fv[2][:, cidx, 0:64], pt2[:, 0:64])

        # --- attention ---
        nch = _nchunks(j)

        def s_slice(cidx):
            if cidx == 0:
                return 0, MT
            return MT + 128 * (cidx - 1), MT + 128 * cidx

        def head_ops(h, acc):
            """emit scores+exp+AV chunk loop for head h accumulating into acc."""
            if h == 0:
                flo, fhi, klo, khi = 0, 64, 0, 64      # fkT01 rows / qT01 rows
                ksrc, qsrc = fkT01, qT01
            elif h == 1:
                flo, fhi, klo, khi = 64, 128, 64, 128
                ksrc, qsrc = fkT01, qT01
            else:
                flo, fhi, klo, khi = 64, 128, 64, 128
                ksrc, qsrc = q2k2, v2q2
            for cidx in range(nch):
                slo, shi = s_slice(cidx)
                m = shi - slo
                st = psX.tile([128, 512], F32, tag="ps")
                nc.tensor.matmul(st[0:m, :], ksrc[flo:fhi, slo:shi],
                                 qsrc[klo:khi, q0:q1], start=True, stop=True)
                et = expp.tile([128, 512], F32R, tag="et")
                k = cidx - 4 * j
                if cidx > 0 and k >= 1:
                    qv = 128 * (k - 1)        # first valid local q col
                    if qv > 0:
                        nc.gpsimd.memset(et[0:m, 0:qv].bitcast(F32), 0.0)
                    nc.scalar.activation(et[0:m, qv:512], st[0:m, qv:512], AF.Exp)
                    nc.gpsimd.tensor_tensor(out=et[0:m, qv:512], in0=et[0:m, qv:512],
                                            in1=m01[0:m, 0:512 - qv], op=OP.mult)
                else:
                    nc.scalar.activation(et[0:m, :], st[0:m, :], AF.Exp)
                if dbg and h == 0 and j == 0 and cidx == 1:
                    nc.sync.dma_start(out=dbg["d_et0"][:, :], in_=et.bitcast(F32))
                nc.tensor.matmul(acc[0:65, :], fv[h][0:m, cidx, :], et[0:m, :],
                                 start=(cidx == 0), stop=(cidx == nch - 1))

        def normalize(h, acc):
            dstg = smalls.tile([1, 512], F32, tag="dstg")
            nc.vector.tensor_copy(dstg, acc[64:65, :])
            den = smalls.tile([1, 512], F32, tag="den")
            nc.vector.reciprocal_approx_fast(den, dstg)
            bc = smalls.tile([64, 512], F32, tag="bc")
            nc.gpsimd.partition_broadcast(bc, den, channels=64)
            if dbg and h == 0 and j == 0:
                st0 = outst.tile([128, 512], F32, tag="ot")
                nc.vector.tensor_copy(st0[0:65, :], acc[0:65, :])
                nc.sync.dma_start(out=dbg["d_acc0"][:, :], in_=st0)
                nc.sync.dma_start(out=dbg["d_den0"][:, :], in_=den)
                nc.sync.dma_start(out=dbg["d_bc0"][:, :], in_=bc)
            if h == 0:
                dst = yA[0:64, q0:q1]
            elif h == 1:
                dst = yA[64:128, q0:q1]
            else:
                dst = yB[0:64, q0:q1]
            nc.vector.tensor_tensor(out=dst, in0=acc[0:64, :], in1=bc, op=OP.mult)

        acc0 = psV.tile([128, 512], F32, tag="pv")
        acc1 = psV.tile([128, 512], F32, tag="pv")
        head_ops(0, acc0)
        head_ops(1, acc1)
        normalize(0, acc0)
        normalize(1, acc1)
        acc2 = psV.tile([128, 512], F32, tag="pv")
        head_ops(2, acc2)
        normalize(2, acc2)

        # --- output projection for T-chunks of this q-tile ---
        for t in range(4 * j, 4 * (j + 1)):
            t0, t1 = 128 * t, 128 * (t + 1)
            for (n0, n1) in ((0, 512), (512, 768)):
                pt = psA.tile([128, 512], F32, tag="pa")
                nc.tensor.matmul(pt[:, 0:n1 - n0], yA[:, t0:t1], wproj[:, 0, n0:n1],
                                 start=True, stop=False)
                nc.tensor.matmul(pt[:, 0:n1 - n0], yB[0:65, t0:t1], wproj[0:65, 1, n0:n1],
                                 start=False, stop=True)
                ot = outst.tile([128, 512], F32, tag="ot")
                nc.vector.tensor_copy(ot[:, 0:n1 - n0], pt[:, 0:n1 - n0])
                nc.sync.dma_start(out=out_d[t0:t1, n0:n1], in_=ot[:, 0:n1 - n0])

    if dbg:
        for name, src in (("d_xT", xT), ("d_qT01", qT01), ("d_fkT01", fkT01),
                          ("d_q2k2", q2k2), ("d_v01T", v01T), ("d_v2q2", v2q2),
                          ("d_fv0", fv[0]), ("d_fv2", fv[2]), ("d_yA", yA),
                          ("d_yB", yB), ("d_m01", m01), ("d_memT", memT)):
            flat = src
            if len(src.shape) == 3:
                flat = src.rearrange("p a b -> p (a b)")
            nc.sync.dma_start(out=dbg[name][:, :], in_=flat.bitcast(F32))


# ---------------- host side ----------------

_NC = None


def _get_nc():
    global _NC
    if _NC is None:
        _NC = build_nc()
    return _NC


def _shard_inputs(mem, x, Wqkv, bqkv, Wmem, bmem, Wproj, bproj):
    """Build the 8 per-core input maps."""
    f32 = np.float32
    mem, x = np.asarray(mem, f32), np.asarray(x, f32)
    Wqkv, bqkv = np.asarray(Wqkv, f32), np.asarray(bqkv, f32)
    Wmem, bmem = np.asarray(Wmem, f32), np.asarray(bmem, f32)
    Wproj, bproj = np.asarray(Wproj, f32), np.asarray(bproj, f32)

    in_maps = []
    for core in range(8):
        b, g = core // 4, core % 4
        hs = [HPC * g + i for i in range(HPC)]     # global head ids
        # q/k/v column slices in Wqkv: q block [0,C), k [C,2C), v [2C,3C); head h -> cols h*HD..
        def qc(h):
            return slice(HD * h, HD * (h + 1))
        def kc(h):
            return slice(C + HD * h, C + HD * (h + 1))
        def vc(h):
            return slice(2 * C + HD * h, 2 * C + HD * (h + 1))
        h0, h1, h2 = hs
        sc = np.float32(1.0 / np.sqrt(HD))         # fold score scale into q
        wq = np.concatenate([
            sc * Wqkv[:, qc(h0)], sc * Wqkv[:, qc(h1)],   # chunk0 [q0|q1]
            Wqkv[:, kc(h0)], Wqkv[:, kc(h1)],             # chunk1 [k0|k1]
            sc * Wqkv[:, qc(h2)], Wqkv[:, kc(h2)],        # chunk2 [q2|k2]
            Wqkv[:, vc(h0)], Wqkv[:, vc(h1)],             # chunk3 [v0|v1]
            Wqkv[:, vc(h2)], sc * Wqkv[:, qc(h2)],        # chunk4 [v2|q2dup]
        ], axis=1)
        bq = np.concatenate([
            sc * bqkv[qc(h0)], sc * bqkv[qc(h1)], bqkv[kc(h0)], bqkv[kc(h1)],
            sc * bqkv[qc(h2)], bqkv[kc(h2)], bqkv[vc(h0)], bqkv[vc(h1)],
            bqkv[vc(h2)], sc * bqkv[qc(h2)],
        ])[None, :]
        wm = np.concatenate([
            Wmem[:, kc(h0)], Wmem[:, kc(h1)],                    # k01 chunk
            np.zeros((C, HD), f32), Wmem[:, kc(h2)],             # [pad|k2]
            Wmem[:, vc(h0)], Wmem[:, vc(h1)], Wmem[:, vc(h2)],   # v3
        ], axis=1)
        bmk = np.concatenate([
            bmem[kc(h0)], bmem[kc(h1)], np.zeros(HD, f32), bmem[kc(h2)],
        ])[None, :]
        bmv = np.concatenate([bmem[vc(h0)], bmem[vc(h1)], bmem[vc(h2)]])[None, :]
        # wproj rows for these heads + bias row (bias only on g==0)
        wp = np.concatenate([
            Wproj[HD * h0:HD * (h0 + 1), :], Wproj[HD * h1:HD * (h1 + 1), :],
            Wproj[HD * h2:HD * (h2 + 1), :],
            (bproj[None, :] if g == 0 else np.zeros((1, C), f32)),
        ], axis=0)
        in_maps.append({
            "x": np.ascontiguousarray(x[b]),
            "mem": np.ascontiguousarray(mem[b]),
            "wqkv": np.ascontiguousarray(wq),
            "bqkv": np.ascontiguousarray(bq),
            "wmem": np.ascontiguousarray(wm),
            "bmemk": np.ascontiguousarray(bmk),
            "bmemv": np.ascontiguousarray(bmv),
            "wproj": np.ascontiguousarray(wp),
        })
    return in_maps


def run_on_hw(in_maps, trace=False):
    from concourse.bass_utils import run_bass_kernel_spmd
    nc = _get_nc()
    res = run_bass_kernel_spmd(nc, in_maps, core_ids=list(range(8)), trace=trace)
    return res


def kernel(mem, x, Wqkv, bqkv, Wmem, bmem, Wproj, bproj):
    in_maps = _shard_inputs(mem, x, Wqkv, bqkv, Wmem, bmem, Wproj, bproj)
    trace = bool(int(os.environ.get("KERNEL_TRACE", "0")))
    res = run_on_hw(in_maps, trace=trace)
    if trace:
        kernel.last_exec_time_ns = res.exec_time_ns
    out = np.zeros((B, T, C), np.float32)
    for core in range(8):
        out[core // 4] += res.results[core]["out"]
    return out


# revision 25
# speedup vs baseline: 1.1740x; 1.1740x over previous
"""Bass/Trainium2 kernel for memory-augmented causal self-attention.

Sharding (hardcoded): 8 cores = 2 batches x 4 head-groups (3 heads each).
Each core computes its batch's attention for its 3 heads plus the
row-sharded output projection partial; the host sums the 4 partials.

Self-contained: no reads of reference.py/spec.json.
"""
import os
import sys

for _p in ("/opt/trn_rl_repo",):
    if _p not in sys.path:
        sys.path.append(_p)

import numpy as np
import concourse.bass as bass
import concourse.tile as tile
from concourse import bacc, mybir
from concourse.masks import make_identity

F32 = mybir.dt.float32
F32R = mybir.dt.float32r
AF = mybir.ActivationFunctionType
OP = mybir.AluOpType

# Problem shapes (hardcoded from the task spec)
B, T, C = 2, 2048, 768
H, HD = 12, 64
MT = 64                  # memory tokens
HPC = 3                  # heads per core
S = MT + T               # 2112 keys
NJ = T // 512            # 4 q-tiles of 512
NT = T // 128            # 16 T-chunks of 128
KC = C // 128            # 6 contraction chunks

# qkv projection M-chunk layout (5 chunks of 128 output channels):
#   chunk0 = [q0|q1]  chunk1 = [k0|k1]  chunk2 = [q2|k2]
#   chunk3 = [v0|v1]  chunk4 = [v2|q2dup]
NEG = -1e30


def _nchunks(j):
    # S-chunks for q-tile j: chunk 0 = mem (64 rows), chunks 1..4(j+1) = v blocks
    return 4 * (j + 1) + 1


DEBUG = bool(int(os.environ.get("KERNEL_DEBUG", "0")))


def build_nc():
    nc = bacc.Bacc()
    x_d = nc.declare_dram_parameter("x", [T, C], F32, isOutput=False)
    mem_d = nc.declare_dram_parameter("mem", [MT, C], F32, isOutput=False)
    wqkv_d = nc.declare_dram_parameter("wqkv", [C, 640], F32, isOutput=False)
    bqkv_d = nc.declare_dram_parameter("bqkv", [1, 640], F32, isOutput=False)
    wmem_d = nc.declare_dram_parameter("wmem", [C, 448], F32, isOutput=False)
    bmemk_d = nc.declare_dram_parameter("bmemk", [1, 256], F32, isOutput=False)
    bmemv_d = nc.declare_dram_parameter("bmemv", [1, 192], F32, isOutput=False)
    wproj_d = nc.declare_dram_parameter("wproj", [193, C], F32, isOutput=False)
    out_d = nc.declare_dram_parameter("out", [T, C], F32, isOutput=True)
    dbg = {}
    if DEBUG:
        for name, shape in (("d_xT", [128, KC * T]), ("d_qT01", [128, T]),
                            ("d_fkT01", [128, S]), ("d_q2k2", [128, S]),
                            ("d_v01T", [128, T]), ("d_v2q2", [128, T]),
                            ("d_fv0", [128, 17 * 65]), ("d_fv2", [128, 17 * 65]),
                            ("d_yA", [128, T]), ("d_yB", [128, T]),
                            ("d_m01", [128, 512]), ("d_memT", [128, KC * MT]),
                            ("d_acc0", [128, 512]), ("d_den0", [1, 512]),
                            ("d_bc0", [64, 512]), ("d_et0", [128, 512])):
            dbg[name] = nc.declare_dram_parameter(name, shape, F32, isOutput=True)

    with tile.TileContext(nc) as tc:
        import contextlib
        with contextlib.ExitStack() as ctx:
            _build_tile(ctx, tc, nc, x_d, mem_d, wqkv_d, bqkv_d, wmem_d,
                        bmemk_d, bmemv_d, wproj_d, out_d, dbg)
    nc.finalize()
    return nc


def _build_tile(ctx, tc, nc, x_d, mem_d, wqkv_d, bqkv_d, wmem_d,
                bmemk_d, bmemv_d, wproj_d, out_d, dbg=None):
    # ---- pools ----
    big = ctx.enter_context(tc.tile_pool(name="big", bufs=1))       # persistents
    stage = ctx.enter_context(tc.tile_pool(name="stage", bufs=2))   # x staging
    expp = ctx.enter_context(tc.tile_pool(name="expp", bufs=4))     # exp tiles
    smalls = ctx.enter_context(tc.tile_pool(name="smalls", bufs=2)) # denom/recip
    outst = ctx.enter_context(tc.tile_pool(name="outst", bufs=3))   # out staging
    psA = ctx.enter_context(tc.tile_pool(name="psA", bufs=2, space="PSUM"))
    psX = ctx.enter_context(tc.tile_pool(name="psX", bufs=3, space="PSUM"))
    psV = ctx.enter_context(tc.tile_pool(name="psV", bufs=3, space="PSUM"))

    # ---- persistent SBUF ----
    xT = big.tile([128, KC, T], F32R, tag="xT")          # x^T, chunk c = [:, c, :]
    memT = big.tile([128, KC, MT], F32R, tag="memT")
    wqkv = big.tile([128, KC, 640], F32R, tag="wqkv")
    wmem = big.tile([128, KC, 448], F32R, tag="wmem")
    wproj = big.tile([128, 2, C], F32R, tag="wproj")
    bqkv = big.tile([128, 5], F32, tag="bqkv")
    bmemk = big.tile([128, 2], F32, tag="bmemk")
    bmemv = big.tile([1, 192], F32R, tag="bmemv")
    qT01 = big.tile([128, T], F32R, tag="qT01")          # rows 0:64 q_h0, 64:128 q_h1
    fkT01 = big.tile([128, S], F32R, tag="fkT01")        # [mk|k] same row split
    q2k2 = big.tile([128, S], F32R, tag="q2k2")          # rows 0:64 = qT2 (cols 64:S); rows 64:128 = fkT2
    v01T = big.tile([128, T], F32R, tag="v01T")
    v2q2 = big.tile([128, T], F32R, tag="v2q2")          # rows 0:64 vT2, 64:128 qT2 dup
    # fv per head: [128, chunk, 65]; col 64 = ones (denominator trick)
    fv = [big.tile([128, 17, 65], F32R, tag=f"fv{h}", name=f"fv{h}") for h in range(HPC)]
    yA = big.tile([128, T], F32R, tag="yA")              # rows 0:64 h0, 64:128 h1 (normalized y^T)
    yB = big.tile([128, T], F32R, tag="yB")              # rows 0:64 h2, row 64 = ones (bias row)
    ident = big.tile([128, 128], F32R, tag="ident")
    m01 = big.tile([128, 512], F32R, tag="m01")          # causal multiplier m01[p,i] = p<=i
    ones1 = big.tile([1, 64], F32R, tag="ones1")

    # ---- constants ----
    ident_stg = stage.tile([128, 128], F32, tag="identstg")
    make_identity(nc, ident_stg)
    nc.vector.tensor_copy(ident, ident_stg)
    # m01[p, i] = (p - i) <= 0 ? in_(1.0) : fill(0.0)
    nc.gpsimd.memset(m01.bitcast(F32), 1.0)
    # expr = i - p >= 0  <=>  p <= i
    nc.gpsimd.affine_select(out=m01.bitcast(F32), in_=m01.bitcast(F32),
                            compare_op=OP.is_ge, fill=0.0, base=0,
                            pattern=[[1, 512]], channel_multiplier=-1)
    nc.gpsimd.memset(ones1.bitcast(F32), 1.0)
    for h in range(HPC):
        nc.gpsimd.memset(fv[h][:, :, 64:65].bitcast(F32), 1.0)
    nc.gpsimd.memset(yB[64:65, :].bitcast(F32), 1.0)

    # ---- weight / bias DMAs ----
    nc.sync.dma_start(out=wqkv, in_=wqkv_d[:, :].bitcast(F32R).rearrange("(c p) m -> p c m", p=128))
    nc.sync.dma_start(out=wmem, in_=wmem_d[:, :].bitcast(F32R).rearrange("(c p) m -> p c m", p=128))
    nc.sync.dma_start(out=wproj[:, 0, :], in_=wproj_d[0:128, :].bitcast(F32R))
    nc.sync.dma_start(out=wproj[0:65, 1, :], in_=wproj_d[128:193, :].bitcast(F32R))
    nc.sync.dma_start(out=bqkv, in_=bqkv_d[0, :].rearrange("(m p) -> p m", p=128))
    nc.sync.dma_start(out=bmemk, in_=bmemk_d[0, :].rearrange("(m p) -> p m", p=128))
    nc.sync.dma_start(out=bmemv, in_=bmemv_d[:, :].bitcast(F32R))

    # ---- mem: load, transpose, project ----
    mstg = stage.tile([MT, C], F32R, tag="mstg")
    nc.sync.dma_start(out=mstg, in_=mem_d[:, :].bitcast(F32R))
    for c in range(KC):
        pt = psX.tile([128, 512], F32R, tag="ps")
        nc.tensor.transpose(pt[:, 0:MT], mstg[:, 128 * c:128 * (c + 1)], ident[0:MT, 0:MT])
        nc.vector.tensor_copy(memT[:, c, :], pt[:, 0:MT])
    # mk^T chunks -> fkT cols 0:MT.  chunk0=[mk0|mk1] -> fkT01; chunk1=[pad|mk2] -> q2k2 rows 64:128
    for mchunk, (dst, rows) in enumerate(((fkT01, slice(0, 128)), (q2k2, slice(64, 128)))):
        pt = psX.tile([128, 512], F32, tag="ps")
        for c in range(KC):
            nc.tensor.matmul(pt[:, 0:MT], wmem[:, c, 128 * mchunk:128 * (mchunk + 1)],
                             memT[:, c, :], start=(c == 0), stop=(c == KC - 1))
        nc.vector.tensor_scalar(out=dst[rows, 0:MT], in0=pt[rows, 0:MT],
                                scalar1=bmemk[rows, mchunk:mchunk + 1], scalar2=None,
                                op0=OP.add)
    # mv natural [tokens, dims] for fv chunk 0 (+ bias via K=1 ones row)
    pt = psX.tile([128, 512], F32, tag="ps")
    for c in range(KC):
        nc.tensor.matmul(pt[0:MT, 0:192], memT[:, c, :], wmem[:, c, 256:448],
                         start=(c == 0), stop=False)
    nc.tensor.matmul(pt[0:MT, 0:192], ones1, bmemv, start=False, stop=True)
    for h in range(HPC):
        nc.vector.tensor_copy(fv[h][0:MT, 0, 0:64], pt[0:MT, 64 * h:64 * (h + 1)])

    # ---- x: load + transpose ----
    for t in range(NT):
        xs = stage.tile([128, C], F32R, tag="xs")
        nc.sync.dma_start(out=xs, in_=x_d[128 * t:128 * (t + 1), :].bitcast(F32R))
        for c in range(KC):
            pt = psX.tile([128, 512], F32R, tag="ps")
            nc.tensor.transpose(pt[:, 0:128], xs[:, 128 * c:128 * (c + 1)], ident)
            nc.vector.tensor_copy(xT[:, c, 128 * t:128 * (t + 1)], pt[:, 0:128])

    # ---- qkv projection + attention + output projection, per q-tile j ----
    # proj M-chunk destinations: (tensor, col offset in that tensor)
    def proj_dst(m):
        return ((qT01, 0), (fkT01, MT), (q2k2, MT), (v01T, 0), (v2q2, 0))[m]

    for j in range(NJ):
        q0, q1 = 512 * j, 512 * (j + 1)
        # --- projection for this q-tile: 5 M-chunks x 6 K matmuls ---
        for m in range(5):
            pt = psA.tile([128, 512], F32, tag="pa")
            for c in range(KC):
                nc.tensor.matmul(pt, wqkv[:, c, 128 * m:128 * (m + 1)],
                                 xT[:, c, q0:q1], start=(c == 0), stop=(c == KC - 1))
            dst, off = proj_dst(m)
            nc.vector.tensor_scalar(out=dst[:, off + q0:off + q1], in0=pt,
                                    scalar1=bqkv[:, m:m + 1], scalar2=None, op0=OP.add)
        # --- v transposes for blocks 4j..4j+3 -> fv chunks 4j+1..4j+4 ---
        for b in range(4 * j, 4 * (j + 1)):
            cidx = b + 1
            pt = psX.tile([128, 512], F32R, tag="ps")
            nc.tensor.transpose(pt[:, 0:128], v01T[:, 128 * b:128 * (b + 1)], ident)
            nc.vector.tensor_copy(fv[0][:, cidx, 0:64], pt[:, 0:64])
            nc.vector.tensor_copy(fv[1][:, cidx, 0:64], pt[:, 64:128])
            pt2 = psX.tile([128, 512], F32R, tag="ps")
            nc.tensor.transpose(pt2[:, 0:64], v2q2[0:64, 128 * b:128 * (b + 1)], ident[0:64, 0:64])
            nc.vector.tensor_copy(fv[2][:, cidx, 0:64], pt2[:, 0:64])

        # --- attention ---
        nch = _nchunks(j)

        def s_slice(cidx):
            if cidx == 0:
                return 0, MT
            return MT + 128 * (cidx - 1), MT + 128 * cidx

        # per-head operand sources: (k-rows lo/hi, q-rows lo/hi, k tensor, q tensor)
        HSRC = ((0, 64, 0, 64, fkT01, qT01),
                (64, 128, 64, 128, fkT01, qT01),
                (64, 128, 64, 128, q2k2, v2q2))

        def emit_scores(h, cidx):
            flo, fhi, klo, khi, ksrc, qsrc = HSRC[h]
            slo, shi = s_slice(cidx)
            m = shi - slo
            st = psX.tile([128, 512], F32, tag="ps", name=f"st{h}")
            nc.tensor.matmul(st[0:m, :], ksrc[flo:fhi, slo:shi],
                             qsrc[klo:khi, q0:q1], start=True, stop=True)
            et = expp.tile([128, 512], F32R, tag="et", name=f"et{h}")
            k = cidx - 4 * j
            if cidx > 0 and k >= 1:
                qv = 128 * (k - 1)            # first valid local q col
                if qv > 0:
                    nc.gpsimd.memset(et[0:m, 0:qv].bitcast(F32), 0.0)
                nc.scalar.activation(et[0:m, qv:512], st[0:m, qv:512], AF.Exp)
                nc.gpsimd.tensor_tensor(out=et[0:m, qv:512], in0=et[0:m, qv:512],
                                        in1=m01[0:m, 0:512 - qv], op=OP.mult)
            else:
                nc.scalar.activation(et[0:m, :], st[0:m, :], AF.Exp)
            return et

        def emit_av(h, cidx, acc, et):
            slo, shi = s_slice(cidx)
            m = shi - slo
            nc.tensor.matmul(acc[0:65, :], fv[h][0:m, cidx, :], et[0:m, :],
                             start=(cidx == 0), stop=(cidx == nch - 1))

        def normalize(h, acc):
            dstg = smalls.tile([1, 512], F32, tag="dstg")
            nc.vector.tensor_copy(dstg, acc[64:65, :])
            den = smalls.tile([1, 512], F32, tag="den")
            nc.vector.reciprocal_approx_fast(den, dstg)
            bc = smalls.tile([64, 512], F32, tag="bc")
            nc.gpsimd.partition_broadcast(bc, den, channels=64)
            if dbg and h == 0 and j == 0:
                st0 = outst.tile([128, 512], F32, tag="ot")
                nc.vector.tensor_copy(st0[0:65, :], acc[0:65, :])
                nc.sync.dma_start(out=dbg["d_acc0"][:, :], in_=st0)
                nc.sync.dma_start(out=dbg["d_den0"][:, :], in_=den)
                nc.sync.dma_start(out=dbg["d_bc0"][:, :], in_=bc)
            if h == 0:
                dst = yA[0:64, q0:q1]
            elif h == 1:
                dst = yA[64:128, q0:q1]
            else:
                dst = yB[0:64, q0:q1]
            nc.vector.tensor_tensor(out=dst, in0=acc[0:64, :], in1=bc, op=OP.mult)

        accs = [psV.tile([128, 512], F32, tag="pv", name=f"acc{h}") for h in range(HPC)]
        for cidx in range(nch):
            ets = [emit_scores(h, cidx) for h in range(HPC)]
            for h in range(HPC):
                emit_av(h, cidx, accs[h], ets[h])
        for h in range(HPC):
            normalize(h, accs[h])

        # --- output projection for T-chunks of this q-tile ---
        for t in range(4 * j, 4 * (j + 1)):
            t0, t1 = 128 * t, 128 * (t + 1)
            for (n0, n1) in ((0, 512), (512, 768)):
                pt = psA.tile([128, 512], F32, tag="pa")
                nc.tensor.matmul(pt[:, 0:n1 - n0], yA[:, t0:t1], wproj[:, 0, n0:n1],
                                 start=True, stop=False)
                nc.tensor.matmul(pt[:, 0:n1 - n0], yB[0:65, t0:t1], wproj[0:65, 1, n0:n1],
                                 start=False, stop=True)
                ot = outst.tile([128, 512], F32, tag="ot")
                nc.vector.tensor_copy(ot[:, 0:n1 - n0], pt[:, 0:n1 - n0])
                nc.sync.dma_start(out=out_d[t0:t1, n0:n1], in_=ot[:, 0:n1 - n0])

    if dbg:
        for name, src in (("d_xT", xT), ("d_qT01", qT01), ("d_fkT01", fkT01),
                          ("d_q2k2", q2k2), ("d_v01T", v01T), ("d_v2q2", v2q2),
                          ("d_fv0", fv[0]), ("d_fv2", fv[2]), ("d_yA", yA),
                          ("d_yB", yB), ("d_m01", m01), ("d_memT", memT)):
            flat = src
            if len(src.shape) == 3:
                flat = src.rearrange("p a b -> p (a b)")
            nc.sync.dma_start(out=dbg[name][:, :], in_=flat.bitcast(F32))


# ---------------- host side ----------------

_NC = None


def _get_nc():
    global _NC
    if _NC is None:
        _NC = build_nc()
    return _NC


def _shard_inputs(mem, x, Wqkv, bqkv, Wmem, bmem, Wproj, bproj):
    """Build the 8 per-core input maps."""
    f32 = np.float32
    mem, x = np.asarray(mem, f32), np.asarray(x, f32)
    Wqkv, bqkv = np.asarray(Wqkv, f32), np.asarray(bqkv, f32)
    Wmem, bmem = np.asarray(Wmem, f32), np.asarray(bmem, f32)
    Wproj, bproj = np.asarray(Wproj, f32), np.asarray(bproj, f32)

    in_maps = []
    for core in range(8):
        b, g = core // 4, core % 4
        hs = [HPC * g + i for i in range(HPC)]     # global head ids
        # q/k/v column slices in Wqkv: q block [0,C), k [C,2C), v [2C,3C); head h -> cols h*HD..
        def qc(h):
            return slice(HD * h, HD * (h + 1))
        def kc(h):
            return slice(C + HD * h, C + HD * (h + 1))
        def vc(h):
            return slice(2 * C + HD * h, 2 * C + HD * (h + 1))
        h0, h1, h2 = hs
        sc = np.float32(1.0 / np.sqrt(HD))         # fold score scale into q
        wq = np.concatenate([
            sc * Wqkv[:, qc(h0)], sc * Wqkv[:, qc(h1)],   # chunk0 [q0|q1]
            Wqkv[:, kc(h0)], Wqkv[:, kc(h1)],             # chunk1 [k0|k1]
            sc * Wqkv[:, qc(h2)], Wqkv[:, kc(h2)],        # chunk2 [q2|k2]
            Wqkv[:, vc(h0)], Wqkv[:, vc(h1)],             # chunk3 [v0|v1]
            Wqkv[:, vc(h2)], sc * Wqkv[:, qc(h2)],        # chunk4 [v2|q2dup]
        ], axis=1)
        bq = np.concatenate([
            sc * bqkv[qc(h0)], sc * bqkv[qc(h1)], bqkv[kc(h0)], bqkv[kc(h1)],
            sc * bqkv[qc(h2)], bqkv[kc(h2)], bqkv[vc(h0)], bqkv[vc(h1)],
            bqkv[vc(h2)], sc * bqkv[qc(h2)],
        ])[None, :]
        wm = np.concatenate([
            Wmem[:, kc(h0)], Wmem[:, kc(h1)],                    # k01 chunk
            np.zeros((C, HD), f32), Wmem[:, kc(h2)],             # [pad|k2]
            Wmem[:, vc(h0)], Wmem[:, vc(h1)], Wmem[:, vc(h2)],   # v3
        ], axis=1)
        bmk = np.concatenate([
            bmem[kc(h0)], bmem[kc(h1)], np.zeros(HD, f32), bmem[kc(h2)],
        ])[None, :]
        bmv = np.concatenate([bmem[vc(h0)], bmem[vc(h1)], bmem[vc(h2)]])[None, :]
        # wproj rows for these heads + bias row (bias only on g==0)
        wp = np.concatenate([
            Wproj[HD * h0:HD * (h0 + 1), :], Wproj[HD * h1:HD * (h1 + 1), :],
            Wproj[HD * h2:HD * (h2 + 1), :],
            (bproj[None, :] if g == 0 else np.zeros((1, C), f32)),
        ], axis=0)
        in_maps.append({
            "x": np.ascontiguousarray(x[b]),
            "mem": np.ascontiguousarray(mem[b]),
            "wqkv": np.ascontiguousarray(wq),
            "bqkv": np.ascontiguousarray(bq),
            "wmem": np.ascontiguousarray(wm),
            "bmemk": np.ascontiguousarray(bmk),
            "bmemv": np.ascontiguousarray(bmv),
            "wproj": np.ascontiguousarray(wp),
        })
    return in_maps


def run_on_hw(in_maps, trace=False):
    from concourse.bass_utils import run_bass_kernel_spmd
    nc = _get_nc()
    res = run_bass_kernel_spmd(nc, in_maps, core_ids=list(range(8)), trace=trace)
    return res


def kernel(mem, x, Wqkv, bqkv, Wmem, bmem, Wproj, bproj):
    in_maps = _shard_inputs(mem, x, Wqkv, bqkv, Wmem, bmem, Wproj, bproj)
    trace = bool(int(os.environ.get("KERNEL_TRACE", "0")))
    res = run_on_hw(in_maps, trace=trace)
    if trace:
        kernel.last_exec_time_ns = res.exec_time_ns
    out = np.zeros((B, T, C), np.float32)
    for core in range(8):
        out[core // 4] += res.results[core]["out"]
    return out


# revision 26
# speedup vs baseline: 1.2546x; 1.0686x over previous
"""Bass/Trainium2 kernel for memory-augmented causal self-attention.

Sharding (hardcoded): 8 cores = 2 batches x 4 head-groups (3 heads each).
Each core computes its batch's attention for its 3 heads plus the
row-sharded output projection partial; the host sums the 4 partials.

Self-contained: no reads of reference.py/spec.json.
"""
import os
import sys

for _p in ("/opt/trn_rl_repo",):
    if _p not in sys.path:
        sys.path.append(_p)

import numpy as np
import concourse.bass as bass
import concourse.tile as tile
from concourse import bacc, mybir
from concourse.masks import make_identity

F32 = mybir.dt.float32
F32R = mybir.dt.float32r
BF16 = mybir.dt.bfloat16
AF = mybir.ActivationFunctionType
OP = mybir.AluOpType

# Problem shapes (hardcoded from the task spec)
B, T, C = 2, 2048, 768
H, HD = 12, 64
MT = 64                  # memory tokens
HPC = 3                  # heads per core
S = MT + T               # 2112 keys
NJ = T // 512            # 4 q-tiles of 512
NT = T // 128            # 16 T-chunks of 128
KC = C // 128            # 6 contraction chunks

# qkv projection M-chunk layout (5 chunks of 128 output channels):
#   chunk0 = [q0|q1]  chunk1 = [k0|k1]  chunk2 = [q2|k2]
#   chunk3 = [v0|v1]  chunk4 = [v2|q2dup]
NEG = -1e30


def _nchunks(j):
    # S-chunks for q-tile j: chunk 0 = mem (64 rows), chunks 1..4(j+1) = v blocks
    return 4 * (j + 1) + 1


DEBUG = bool(int(os.environ.get("KERNEL_DEBUG", "0")))


def build_nc():
    nc = bacc.Bacc()
    x_d = nc.declare_dram_parameter("x", [T, C], F32, isOutput=False)
    mem_d = nc.declare_dram_parameter("mem", [MT, C], F32, isOutput=False)
    wqkv_d = nc.declare_dram_parameter("wqkv", [C, 640], F32, isOutput=False)
    bqkv_d = nc.declare_dram_parameter("bqkv", [1, 640], F32, isOutput=False)
    wmem_d = nc.declare_dram_parameter("wmem", [C, 448], F32, isOutput=False)
    bmemk_d = nc.declare_dram_parameter("bmemk", [1, 256], F32, isOutput=False)
    bmemv_d = nc.declare_dram_parameter("bmemv", [1, 192], F32, isOutput=False)
    wproj_d = nc.declare_dram_parameter("wproj", [193, C], F32, isOutput=False)
    out_d = nc.declare_dram_parameter("out", [T, C], F32, isOutput=True)
    dbg = {}
    if DEBUG:
        for name, shape in (("d_xT", [128, KC * T]), ("d_qT01", [128, T]),
                            ("d_fkT01", [128, S]), ("d_q2k2", [128, S]),
                            ("d_v01T", [128, T]), ("d_v2q2", [128, T]),
                            ("d_fv0", [128, 17 * 65]), ("d_fv2", [128, 17 * 65]),
                            ("d_yA", [128, T]), ("d_yB", [128, T]),
                            ("d_m01", [128, 512]), ("d_memT", [128, KC * MT]),
                            ("d_acc0", [128, 512]), ("d_den0", [1, 512]),
                            ("d_bc0", [64, 512]), ("d_et0", [128, 512])):
            dbg[name] = nc.declare_dram_parameter(name, shape, F32, isOutput=True)

    with tile.TileContext(nc) as tc:
        import contextlib
        with contextlib.ExitStack() as ctx:
            _build_tile(ctx, tc, nc, x_d, mem_d, wqkv_d, bqkv_d, wmem_d,
                        bmemk_d, bmemv_d, wproj_d, out_d, dbg)
    nc.finalize()
    return nc


def _build_tile(ctx, tc, nc, x_d, mem_d, wqkv_d, bqkv_d, wmem_d,
                bmemk_d, bmemv_d, wproj_d, out_d, dbg=None):
    # ---- pools ----
    big = ctx.enter_context(tc.tile_pool(name="big", bufs=1))       # persistents
    stage = ctx.enter_context(tc.tile_pool(name="stage", bufs=2))   # x staging
    expp = ctx.enter_context(tc.tile_pool(name="expp", bufs=4))     # exp tiles
    smalls = ctx.enter_context(tc.tile_pool(name="smalls", bufs=2)) # denom/recip
    outst = ctx.enter_context(tc.tile_pool(name="outst", bufs=3))   # out staging
    psA = ctx.enter_context(tc.tile_pool(name="psA", bufs=2, space="PSUM"))
    psX = ctx.enter_context(tc.tile_pool(name="psX", bufs=3, space="PSUM"))
    psV = ctx.enter_context(tc.tile_pool(name="psV", bufs=3, space="PSUM"))

    # ---- persistent SBUF ----
    xT = big.tile([128, KC, T], F32R, tag="xT")          # x^T, chunk c = [:, c, :]
    memT = big.tile([128, KC, MT], F32R, tag="memT")
    wqkv = big.tile([128, KC, 640], F32R, tag="wqkv")
    wmem = big.tile([128, KC, 448], F32R, tag="wmem")
    wproj = big.tile([128, 2, C], F32R, tag="wproj")
    bqkv = big.tile([128, 5], F32, tag="bqkv")
    bmemk = big.tile([128, 2], F32, tag="bmemk")
    bmemv = big.tile([1, 192], F32R, tag="bmemv")
    qT01 = big.tile([128, T], BF16, tag="qT01")          # rows 0:64 q_h0, 64:128 q_h1
    fkT01 = big.tile([128, S], BF16, tag="fkT01")        # [mk|k] same row split
    q2k2 = big.tile([128, S], BF16, tag="q2k2")          # rows 0:64 = qT2 (cols 64:S); rows 64:128 = fkT2
    v01T = big.tile([128, T], BF16, tag="v01T")
    v2q2 = big.tile([128, T], BF16, tag="v2q2")          # rows 0:64 vT2, 64:128 qT2 dup
    # fv per head: [128, chunk, 65]; col 64 = ones (denominator trick)
    fv = [big.tile([128, 17, 65], BF16, tag=f"fv{h}", name=f"fv{h}") for h in range(HPC)]
    yA = big.tile([128, T], F32R, tag="yA")              # rows 0:64 h0, 64:128 h1 (normalized y^T)
    yB = big.tile([128, T], F32R, tag="yB")              # rows 0:64 h2, row 64 = ones (bias row)
    ident = big.tile([128, 128], F32R, tag="ident")
    m01 = big.tile([128, 512], BF16, tag="m01")          # causal multiplier m01[p,i] = p<=i
    ones1 = big.tile([1, 64], F32R, tag="ones1")
    ident_bf = big.tile([128, 128], BF16, tag="identbf")

    # ---- constants ----
    ident_stg = stage.tile([128, 128], F32, tag="identstg")
    make_identity(nc, ident_stg)
    nc.vector.tensor_copy(ident, ident_stg)
    nc.vector.tensor_copy(ident_bf, ident_stg)
    # m01[p, i] = (p - i) <= 0 ? in_(1.0) : fill(0.0)
    nc.gpsimd.memset(m01, 1.0)
    # expr = i - p >= 0  <=>  p <= i
    nc.gpsimd.affine_select(out=m01, in_=m01,
                            compare_op=OP.is_ge, fill=0.0, base=0,
                            pattern=[[1, 512]], channel_multiplier=-1)
    nc.gpsimd.memset(ones1.bitcast(F32), 1.0)
    for h in range(HPC):
        nc.gpsimd.memset(fv[h][:, :, 64:65], 1.0)
    nc.gpsimd.memset(yB[64:65, :].bitcast(F32), 1.0)

    # ---- weight / bias DMAs ----
    nc.sync.dma_start(out=wqkv, in_=wqkv_d[:, :].bitcast(F32R).rearrange("(c p) m -> p c m", p=128))
    nc.sync.dma_start(out=wmem, in_=wmem_d[:, :].bitcast(F32R).rearrange("(c p) m -> p c m", p=128))
    nc.sync.dma_start(out=wproj[:, 0, :], in_=wproj_d[0:128, :].bitcast(F32R))
    nc.sync.dma_start(out=wproj[0:65, 1, :], in_=wproj_d[128:193, :].bitcast(F32R))
    nc.sync.dma_start(out=bqkv, in_=bqkv_d[0, :].rearrange("(m p) -> p m", p=128))
    nc.sync.dma_start(out=bmemk, in_=bmemk_d[0, :].rearrange("(m p) -> p m", p=128))
    nc.sync.dma_start(out=bmemv, in_=bmemv_d[:, :].bitcast(F32R))

    # ---- mem: load, transpose, project ----
    mstg = stage.tile([MT, C], F32R, tag="mstg")
    nc.sync.dma_start(out=mstg, in_=mem_d[:, :].bitcast(F32R))
    for c in range(KC):
        pt = psX.tile([128, 512], F32R, tag="ps")
        nc.tensor.transpose(pt[:, 0:MT], mstg[:, 128 * c:128 * (c + 1)], ident[0:MT, 0:MT])
        nc.vector.tensor_copy(memT[:, c, :], pt[:, 0:MT])
    # mk^T chunks -> fkT cols 0:MT.  chunk0=[mk0|mk1] -> fkT01; chunk1=[pad|mk2] -> q2k2 rows 64:128
    for mchunk, (dst, rows) in enumerate(((fkT01, slice(0, 128)), (q2k2, slice(64, 128)))):
        pt = psX.tile([128, 512], F32, tag="ps")
        for c in range(KC):
            nc.tensor.matmul(pt[:, 0:MT], wmem[:, c, 128 * mchunk:128 * (mchunk + 1)],
                             memT[:, c, :], start=(c == 0), stop=(c == KC - 1))
        nc.vector.tensor_scalar(out=dst[rows, 0:MT], in0=pt[rows, 0:MT],
                                scalar1=bmemk[rows, mchunk:mchunk + 1], scalar2=None,
                                op0=OP.add)
    # mv natural [tokens, dims] for fv chunk 0 (+ bias via K=1 ones row)
    pt = psX.tile([128, 512], F32, tag="ps")
    for c in range(KC):
        nc.tensor.matmul(pt[0:MT, 0:192], memT[:, c, :], wmem[:, c, 256:448],
                         start=(c == 0), stop=False)
    nc.tensor.matmul(pt[0:MT, 0:192], ones1, bmemv, start=False, stop=True)
    for h in range(HPC):
        nc.vector.tensor_copy(fv[h][0:MT, 0, 0:64], pt[0:MT, 64 * h:64 * (h + 1)])

    # ---- x: load + transpose ----
    for t in range(NT):
        xs = stage.tile([128, C], F32R, tag="xs")
        nc.sync.dma_start(out=xs, in_=x_d[128 * t:128 * (t + 1), :].bitcast(F32R))
        for c in range(KC):
            pt = psX.tile([128, 512], F32R, tag="ps")
            nc.tensor.transpose(pt[:, 0:128], xs[:, 128 * c:128 * (c + 1)], ident)
            nc.vector.tensor_copy(xT[:, c, 128 * t:128 * (t + 1)], pt[:, 0:128])

    # ---- qkv projection + attention + output projection, per q-tile j ----
    # proj M-chunk destinations: (tensor, col offset in that tensor)
    def proj_dst(m):
        return ((qT01, 0), (fkT01, MT), (q2k2, MT), (v01T, 0), (v2q2, 0))[m]

    for j in range(NJ):
        q0, q1 = 512 * j, 512 * (j + 1)
        # --- projection for this q-tile: 5 M-chunks x 6 K matmuls ---
        for m in range(5):
            pt = psA.tile([128, 512], F32, tag="pa")
            for c in range(KC):
                nc.tensor.matmul(pt, wqkv[:, c, 128 * m:128 * (m + 1)],
                                 xT[:, c, q0:q1], start=(c == 0), stop=(c == KC - 1))
            dst, off = proj_dst(m)
            nc.vector.tensor_scalar(out=dst[:, off + q0:off + q1], in0=pt,
                                    scalar1=bqkv[:, m:m + 1], scalar2=None, op0=OP.add)
        # --- v transposes for blocks 4j..4j+3 -> fv chunks 4j+1..4j+4 ---
        for b in range(4 * j, 4 * (j + 1)):
            cidx = b + 1
            pt = psX.tile([128, 512], BF16, tag="ps", name="ptv")
            nc.tensor.transpose(pt[:, 0:128], v01T[:, 128 * b:128 * (b + 1)], ident_bf)
            nc.vector.tensor_copy(fv[0][:, cidx, 0:64], pt[:, 0:64])
            nc.vector.tensor_copy(fv[1][:, cidx, 0:64], pt[:, 64:128])
            pt2 = psX.tile([128, 512], BF16, tag="ps", name="ptv2")
            nc.tensor.transpose(pt2[:, 0:64], v2q2[0:64, 128 * b:128 * (b + 1)], ident_bf[0:64, 0:64])
            nc.vector.tensor_copy(fv[2][:, cidx, 0:64], pt2[:, 0:64])

        # --- attention ---
        nch = _nchunks(j)

        def s_slice(cidx):
            if cidx == 0:
                return 0, MT
            return MT + 128 * (cidx - 1), MT + 128 * cidx

        # per-head operand sources: (k-rows lo/hi, q-rows lo/hi, k tensor, q tensor)
        HSRC = ((0, 64, 0, 64, fkT01, qT01),
                (64, 128, 64, 128, fkT01, qT01),
                (64, 128, 64, 128, q2k2, v2q2))

        def emit_scores(h, cidx):
            flo, fhi, klo, khi, ksrc, qsrc = HSRC[h]
            slo, shi = s_slice(cidx)
            m = shi - slo
            st = psX.tile([128, 512], F32, tag="ps", name=f"st{h}")
            nc.tensor.matmul(st[0:m, :], ksrc[flo:fhi, slo:shi],
                             qsrc[klo:khi, q0:q1], start=True, stop=True)
            et = expp.tile([128, 512], BF16, tag="et", name=f"et{h}")
            k = cidx - 4 * j
            if cidx > 0 and k >= 1:
                qv = 128 * (k - 1)            # first valid local q col
                if qv > 0:
                    nc.gpsimd.memset(et[0:m, 0:qv], 0.0)
                nc.scalar.activation(et[0:m, qv:512], st[0:m, qv:512], AF.Exp)
                nc.gpsimd.tensor_tensor(out=et[0:m, qv:512], in0=et[0:m, qv:512],
                                        in1=m01[0:m, 0:512 - qv], op=OP.mult)
            else:
                nc.scalar.activation(et[0:m, :], st[0:m, :], AF.Exp)
            return et

        def emit_av(h, cidx, acc, et):
            slo, shi = s_slice(cidx)
            m = shi - slo
            nc.tensor.matmul(acc[0:65, :], fv[h][0:m, cidx, :], et[0:m, :],
                             start=(cidx == 0), stop=(cidx == nch - 1))

        def normalize(h, acc):
            dstg = smalls.tile([1, 512], F32, tag="dstg")
            nc.vector.tensor_copy(dstg, acc[64:65, :])
            den = smalls.tile([1, 512], F32, tag="den")
            nc.vector.reciprocal_approx_fast(den, dstg)
            bc = smalls.tile([64, 512], F32, tag="bc")
            nc.gpsimd.partition_broadcast(bc, den, channels=64)
            if dbg and h == 0 and j == 0:
                st0 = outst.tile([128, 512], F32, tag="ot")
                nc.vector.tensor_copy(st0[0:65, :], acc[0:65, :])
                nc.sync.dma_start(out=dbg["d_acc0"][:, :], in_=st0)
                nc.sync.dma_start(out=dbg["d_den0"][:, :], in_=den)
                nc.sync.dma_start(out=dbg["d_bc0"][:, :], in_=bc)
            if h == 0:
                dst = yA[0:64, q0:q1]
            elif h == 1:
                dst = yA[64:128, q0:q1]
            else:
                dst = yB[0:64, q0:q1]
            nc.vector.tensor_tensor(out=dst, in0=acc[0:64, :], in1=bc, op=OP.mult)

        accs = [psV.tile([128, 512], F32, tag="pv", name=f"acc{h}") for h in range(HPC)]
        for cidx in range(nch):
            ets = [emit_scores(h, cidx) for h in range(HPC)]
            for h in range(HPC):
                emit_av(h, cidx, accs[h], ets[h])
        for h in range(HPC):
            normalize(h, accs[h])

        # --- output projection for T-chunks of this q-tile ---
        for t in range(4 * j, 4 * (j + 1)):
            t0, t1 = 128 * t, 128 * (t + 1)
            for (n0, n1) in ((0, 512), (512, 768)):
                pt = psA.tile([128, 512], F32, tag="pa")
                nc.tensor.matmul(pt[:, 0:n1 - n0], yA[:, t0:t1], wproj[:, 0, n0:n1],
                                 start=True, stop=False)
                nc.tensor.matmul(pt[:, 0:n1 - n0], yB[0:65, t0:t1], wproj[0:65, 1, n0:n1],
                                 start=False, stop=True)
                ot = outst.tile([128, 512], F32, tag="ot")
                nc.vector.tensor_copy(ot[:, 0:n1 - n0], pt[:, 0:n1 - n0])
                nc.sync.dma_start(out=out_d[t0:t1, n0:n1], in_=ot[:, 0:n1 - n0])

    if dbg:
        for name, src in (("d_xT", xT), ("d_qT01", qT01), ("d_fkT01", fkT01),
                          ("d_q2k2", q2k2), ("d_v01T", v01T), ("d_v2q2", v2q2),
                          ("d_fv0", fv[0]), ("d_fv2", fv[2]), ("d_yA", yA),
                          ("d_yB", yB), ("d_m01", m01), ("d_memT", memT)):
            flat = src
            if len(src.shape) == 3:
                flat = src.rearrange("p a b -> p (a b)")
            nc.sync.dma_start(out=dbg[name][:, :], in_=flat.bitcast(F32))


# ---------------- host side ----------------

_NC = None


def _get_nc():
    global _NC
    if _NC is None:
        _NC = build_nc()
    return _NC


def _shard_inputs(mem, x, Wqkv, bqkv, Wmem, bmem, Wproj, bproj):
    """Build the 8 per-core input maps."""
    f32 = np.float32
    mem, x = np.asarray(mem, f32), np.asarray(x, f32)
    Wqkv, bqkv = np.asarray(Wqkv, f32), np.asarray(bqkv, f32)
    Wmem, bmem = np.asarray(Wmem, f32), np.asarray(bmem, f32)
    Wproj, bproj = np.asarray(Wproj, f32), np.asarray(bproj, f32)

    in_maps = []
    for core in range(8):
        b, g = core // 4, core % 4
        hs = [HPC * g + i for i in range(HPC)]     # global head ids
        # q/k/v column slices in Wqkv: q block [0,C), k [C,2C), v [2C,3C); head h -> cols h*HD..
        def qc(h):
            return slice(HD * h, HD * (h + 1))
        def kc(h):
            return slice(C + HD * h, C + HD * (h + 1))
        def vc(h):
            return slice(2 * C + HD * h, 2 * C + HD * (h + 1))
        h0, h1, h2 = hs
        sc = np.float32(1.0 / np.sqrt(HD))         # fold score scale into q
        wq = np.concatenate([
            sc * Wqkv[:, qc(h0)], sc * Wqkv[:, qc(h1)],   # chunk0 [q0|q1]
            Wqkv[:, kc(h0)], Wqkv[:, kc(h1)],             # chunk1 [k0|k1]
            sc * Wqkv[:, qc(h2)], Wqkv[:, kc(h2)],        # chunk2 [q2|k2]
            Wqkv[:, vc(h0)], Wqkv[:, vc(h1)],             # chunk3 [v0|v1]
            Wqkv[:, vc(h2)], sc * Wqkv[:, qc(h2)],        # chunk4 [v2|q2dup]
        ], axis=1)
        bq = np.concatenate([
            sc * bqkv[qc(h0)], sc * bqkv[qc(h1)], bqkv[kc(h0)], bqkv[kc(h1)],
            sc * bqkv[qc(h2)], bqkv[kc(h2)], bqkv[vc(h0)], bqkv[vc(h1)],
            bqkv[vc(h2)], sc * bqkv[qc(h2)],
        ])[None, :]
        wm = np.concatenate([
            Wmem[:, kc(h0)], Wmem[:, kc(h1)],                    # k01 chunk
            np.zeros((C, HD), f32), Wmem[:, kc(h2)],             # [pad|k2]
            Wmem[:, vc(h0)], Wmem[:, vc(h1)], Wmem[:, vc(h2)],   # v3
        ], axis=1)
        bmk = np.concatenate([
            bmem[kc(h0)], bmem[kc(h1)], np.zeros(HD, f32), bmem[kc(h2)],
        ])[None, :]
        bmv = np.concatenate([bmem[vc(h0)], bmem[vc(h1)], bmem[vc(h2)]])[None, :]
        # wproj rows for these heads + bias row (bias only on g==0)
        wp = np.concatenate([
            Wproj[HD * h0:HD * (h0 + 1), :], Wproj[HD * h1:HD * (h1 + 1), :],
            Wproj[HD * h2:HD * (h2 + 1), :],
            (bproj[None, :] if g == 0 else np.zeros((1, C), f32)),
        ], axis=0)
        in_maps.append({
            "x": np.ascontiguousarray(x[b]),
            "mem": np.ascontiguousarray(mem[b]),
            "wqkv": np.ascontiguousarray(wq),
            "bqkv": np.ascontiguousarray(bq),
            "wmem": np.ascontiguousarray(wm),
            "bmemk": np.ascontiguousarray(bmk),
            "bmemv": np.ascontiguousarray(bmv),
            "wproj": np.ascontiguousarray(wp),
        })
    return in_maps


def run_on_hw(in_maps, trace=False):
    from concourse.bass_utils import run_bass_kernel_spmd
    nc = _get_nc()
    res = run_bass_kernel_spmd(nc, in_maps, core_ids=list(range(8)), trace=trace)
    return res


def kernel(mem, x, Wqkv, bqkv, Wmem, bmem, Wproj, bproj):
    in_maps = _shard_inputs(mem, x, Wqkv, bqkv, Wmem, bmem, Wproj, bproj)
    trace = bool(int(os.environ.get("KERNEL_TRACE", "0")))
    res = run_on_hw(in_maps, trace=trace)
    if trace:
        kernel.last_exec_time_ns = res.exec_time_ns
    out = np.zeros((B, T, C), np.float32)
    for core in range(8):
        out[core // 4] += res.results[core]["out"]
    return out


# revision 30
# speedup vs baseline: 1.3213x; 1.0532x over previous
"""Bass/Trainium2 kernel for memory-augmented causal self-attention.

Sharding (hardcoded): 8 cores = 2 batches x 4 head-groups (3 heads each).
Each core computes its batch's attention for its 3 heads plus the
row-sharded output projection partial; the host sums the 4 partials.

Self-contained: no reads of reference.py/spec.json.
"""
import os
import sys

for _p in ("/opt/trn_rl_repo",):
    if _p not in sys.path:
        sys.path.append(_p)

import numpy as np
import concourse.bass as bass
import concourse.tile as tile
from concourse import bacc, mybir
from concourse.masks import make_identity

F32 = mybir.dt.float32
F32R = mybir.dt.float32r
BF16 = mybir.dt.bfloat16
AF = mybir.ActivationFunctionType
OP = mybir.AluOpType

# Problem shapes (hardcoded from the task spec)
B, T, C = 2, 2048, 768
H, HD = 12, 64
MT = 64                  # memory tokens
HPC = 3                  # heads per core
S = MT + T               # 2112 keys
NJ = T // 512            # 4 q-tiles of 512
NT = T // 128            # 16 T-chunks of 128
KC = C // 128            # 6 contraction chunks

# qkv projection M-chunk layout (5 chunks of 128 output channels):
#   chunk0 = [q0|q1]  chunk1 = [k0|k1]  chunk2 = [q2|k2]
#   chunk3 = [v0|v1]  chunk4 = [v2|q2dup]
NEG = -1e30


def _nchunks(j):
    # S-chunks for q-tile j: chunk 0 = mem (64 rows), chunks 1..4(j+1) = v blocks
    return 4 * (j + 1) + 1


DEBUG = bool(int(os.environ.get("KERNEL_DEBUG", "0")))


def build_nc():
    nc = bacc.Bacc()
    x_d = nc.declare_dram_parameter("x", [T, C], F32, isOutput=False)
    mem_d = nc.declare_dram_parameter("mem", [MT, C], F32, isOutput=False)
    wqkv_d = nc.declare_dram_parameter("wqkv", [C, 640], F32, isOutput=False)
    bqkv_d = nc.declare_dram_parameter("bqkv", [1, 640], F32, isOutput=False)
    wmem_d = nc.declare_dram_parameter("wmem", [C, 448], F32, isOutput=False)
    bmemk_d = nc.declare_dram_parameter("bmemk", [1, 256], F32, isOutput=False)
    bmemv_d = nc.declare_dram_parameter("bmemv", [1, 192], F32, isOutput=False)
    wproj_d = nc.declare_dram_parameter("wproj", [193, C], F32, isOutput=False)
    out_d = nc.declare_dram_parameter("out", [T, C], F32, isOutput=True)
    dbg = {}
    if DEBUG:
        for name, shape in (("d_xT", [128, KC * T]), ("d_qT01", [128, T]),
                            ("d_fkT01", [128, S]), ("d_q2k2", [128, S]),
                            ("d_v01T", [128, T]), ("d_v2q2", [128, T]),
                            ("d_fv0", [128, 17 * 65]), ("d_fv2", [128, 17 * 65]),
                            ("d_yA", [128, T]), ("d_yB", [128, T]),
                            ("d_m01", [128, 512]), ("d_memT", [128, KC * MT]),
                            ("d_acc0", [128, 512]), ("d_den0", [1, 512]),
                            ("d_bc0", [64, 512]), ("d_et0", [128, 512])):
            dbg[name] = nc.declare_dram_parameter(name, shape, F32, isOutput=True)

    with tile.TileContext(nc) as tc:
        import contextlib
        with contextlib.ExitStack() as ctx:
            _build_tile(ctx, tc, nc, x_d, mem_d, wqkv_d, bqkv_d, wmem_d,
                        bmemk_d, bmemv_d, wproj_d, out_d, dbg)
    nc.finalize()
    return nc


def _build_tile(ctx, tc, nc, x_d, mem_d, wqkv_d, bqkv_d, wmem_d,
                bmemk_d, bmemv_d, wproj_d, out_d, dbg=None):
    # ---- pools ----
    big = ctx.enter_context(tc.tile_pool(name="big", bufs=1))       # persistents
    stage = ctx.enter_context(tc.tile_pool(name="stage", bufs=2))   # x staging
    expp = ctx.enter_context(tc.tile_pool(name="expp", bufs=4))     # exp tiles
    smalls = ctx.enter_context(tc.tile_pool(name="smalls", bufs=2)) # denom/recip
    outst = ctx.enter_context(tc.tile_pool(name="outst", bufs=3))   # out staging
    psA = ctx.enter_context(tc.tile_pool(name="psA", bufs=2, space="PSUM"))
    psX = ctx.enter_context(tc.tile_pool(name="psX", bufs=3, space="PSUM"))
    psV = ctx.enter_context(tc.tile_pool(name="psV", bufs=3, space="PSUM"))

    # ---- persistent SBUF ----
    xT = big.tile([128, KC, T], F32R, tag="xT")          # x^T, chunk c = [:, c, :]
    memT = big.tile([128, KC, MT], F32R, tag="memT")
    wqkv = big.tile([128, KC, 640], F32R, tag="wqkv")
    wmem = big.tile([128, KC, 448], F32R, tag="wmem")
    wproj = big.tile([128, 2, C], F32R, tag="wproj")
    bqkv = big.tile([128, 5], F32, tag="bqkv")
    bmemk = big.tile([128, 2], F32, tag="bmemk")
    bmemv = big.tile([1, 192], F32R, tag="bmemv")
    qT01 = big.tile([128, T], BF16, tag="qT01")          # rows 0:64 q_h0, 64:128 q_h1
    fkT01 = big.tile([128, S], BF16, tag="fkT01")        # [mk|k] same row split
    q2k2 = big.tile([128, S], BF16, tag="q2k2")          # rows 0:64 = qT2 (cols 64:S); rows 64:128 = fkT2
    v01T = big.tile([128, T], BF16, tag="v01T")
    v2q2 = big.tile([128, T], BF16, tag="v2q2")          # rows 0:64 vT2, 64:128 qT2 dup
    # fv per head: [128, chunk, 65]; col 64 = ones (denominator trick)
    fv = [big.tile([128, 17, 65], BF16, tag=f"fv{h}", name=f"fv{h}") for h in range(HPC)]
    yA = big.tile([128, T], F32R, tag="yA")              # rows 0:64 h0, 64:128 h1 (normalized y^T)
    yB = big.tile([128, T], F32R, tag="yB")              # rows 0:64 h2, row 64 = ones (bias row)
    ident = big.tile([128, 128], F32R, tag="ident")
    m01 = big.tile([128, 512], BF16, tag="m01")          # causal multiplier m01[p,i] = p<=i
    ones1 = big.tile([1, 64], F32R, tag="ones1")
    ident_bf = big.tile([128, 128], BF16, tag="identbf")

    # ---- constants ----
    ident_stg = stage.tile([128, 128], F32, tag="identstg")
    make_identity(nc, ident_stg)
    nc.vector.tensor_copy(ident, ident_stg)
    nc.vector.tensor_copy(ident_bf, ident_stg)

    # ---- HAM warmup: dense matmul burst so PE reaches 2.4 GHz. Same
    # stationary operand every time (LDWEIGHTS elided) -> pure MM stream.
    # Output is never read; overlaps the x/weight DMAs.
    NWARM = int(os.environ.get("KERNEL_NWARM", "48"))
    if NWARM:
        wsrc = big.tile([128, 512], BF16, tag="wsrc")
        nc.gpsimd.memset(wsrc, 0.5)
        wps = psV.tile([128, 512], F32, tag="pv", name="warmps")
        for i in range(NWARM):
            nc.tensor.matmul(wps, ident_bf, wsrc, start=True, stop=True)
        wsink = big.tile([128, 512], F32, tag="wsink")
        nc.vector.tensor_copy(wsink, wps)
    # m01[p, i] = (p - i) <= 0 ? in_(1.0) : fill(0.0)
    nc.gpsimd.memset(m01, 1.0)
    # expr = i - p >= 0  <=>  p <= i
    nc.gpsimd.affine_select(out=m01, in_=m01,
                            compare_op=OP.is_ge, fill=0.0, base=0,
                            pattern=[[1, 512]], channel_multiplier=-1)
    nc.gpsimd.memset(ones1.bitcast(F32), 1.0)
    for h in range(HPC):
        nc.gpsimd.memset(fv[h][:, :, 64:65], 1.0)
    nc.gpsimd.memset(yB[64:65, :].bitcast(F32), 1.0)

    # ---- weight / bias DMAs ----
    nc.sync.dma_start(out=wqkv, in_=wqkv_d[:, :].bitcast(F32R).rearrange("(c p) m -> p c m", p=128))
    nc.sync.dma_start(out=wmem, in_=wmem_d[:, :].bitcast(F32R).rearrange("(c p) m -> p c m", p=128))
    nc.sync.dma_start(out=wproj[:, 0, :], in_=wproj_d[0:128, :].bitcast(F32R))
    nc.sync.dma_start(out=wproj[0:65, 1, :], in_=wproj_d[128:193, :].bitcast(F32R))
    nc.sync.dma_start(out=bqkv, in_=bqkv_d[0, :].rearrange("(m p) -> p m", p=128))
    nc.sync.dma_start(out=bmemk, in_=bmemk_d[0, :].rearrange("(m p) -> p m", p=128))
    nc.sync.dma_start(out=bmemv, in_=bmemv_d[:, :].bitcast(F32R))

    # ---- mem: load, transpose, project ----
    mstg = stage.tile([MT, C], F32R, tag="mstg")
    nc.sync.dma_start(out=mstg, in_=mem_d[:, :].bitcast(F32R))
    for c in range(KC):
        pt = psX.tile([128, 512], F32R, tag="ps")
        nc.tensor.transpose(pt[:, 0:MT], mstg[:, 128 * c:128 * (c + 1)], ident[0:MT, 0:MT])
        nc.vector.tensor_copy(memT[:, c, :], pt[:, 0:MT])
    # mk^T chunks -> fkT cols 0:MT.  chunk0=[mk0|mk1] -> fkT01; chunk1=[pad|mk2] -> q2k2 rows 64:128
    for mchunk, (dst, rows) in enumerate(((fkT01, slice(0, 128)), (q2k2, slice(64, 128)))):
        pt = psX.tile([128, 512], F32, tag="ps")
        for c in range(KC):
            nc.tensor.matmul(pt[:, 0:MT], wmem[:, c, 128 * mchunk:128 * (mchunk + 1)],
                             memT[:, c, :], start=(c == 0), stop=(c == KC - 1))
        nc.vector.tensor_scalar(out=dst[rows, 0:MT], in0=pt[rows, 0:MT],
                                scalar1=bmemk[rows, mchunk:mchunk + 1], scalar2=None,
                                op0=OP.add)
    # mv natural [tokens, dims] for fv chunk 0 (+ bias via K=1 ones row)
    pt = psX.tile([128, 512], F32, tag="ps")
    for c in range(KC):
        nc.tensor.matmul(pt[0:MT, 0:192], memT[:, c, :], wmem[:, c, 256:448],
                         start=(c == 0), stop=False)
    nc.tensor.matmul(pt[0:MT, 0:192], ones1, bmemv, start=False, stop=True)
    for h in range(HPC):
        nc.vector.tensor_copy(fv[h][0:MT, 0, 0:64], pt[0:MT, 64 * h:64 * (h + 1)])

    # ---- x: load + transpose ----
    for t in range(NT):
        xs = stage.tile([128, C], F32R, tag="xs")
        nc.sync.dma_start(out=xs, in_=x_d[128 * t:128 * (t + 1), :].bitcast(F32R))
        for c in range(KC):
            pt = psX.tile([128, 512], F32R, tag="ps")
            nc.tensor.transpose(pt[:, 0:128], xs[:, 128 * c:128 * (c + 1)], ident)
            nc.vector.tensor_copy(xT[:, c, 128 * t:128 * (t + 1)], pt[:, 0:128])

    # ---- qkv projection + attention + output projection, per q-tile j ----
    # proj M-chunk destinations: (tensor, col offset in that tensor)
    def proj_dst(m):
        return ((qT01, 0), (fkT01, MT), (q2k2, MT), (v01T, 0), (v2q2, 0))[m]

    def emit_yproj(jj):
        # output projection for the 4 T-chunks of q-tile jj
        for t in range(4 * jj, 4 * (jj + 1)):
            t0, t1 = 128 * t, 128 * (t + 1)
            for (n0, n1) in ((0, 512), (512, 768)):
                pt = psA.tile([128, 512], F32, tag="pa", name="yp")
                nc.tensor.matmul(pt[:, 0:n1 - n0], yA[:, t0:t1], wproj[:, 0, n0:n1],
                                 start=True, stop=False)
                nc.tensor.matmul(pt[:, 0:n1 - n0], yB[0:65, t0:t1], wproj[0:65, 1, n0:n1],
                                 start=False, stop=True)
                ot = outst.tile([128, 512], F32, tag="ot")
                nc.vector.tensor_copy(ot[:, 0:n1 - n0], pt[:, 0:n1 - n0])
                nc.sync.dma_start(out=out_d[t0:t1, n0:n1], in_=ot[:, 0:n1 - n0])

    for j in range(NJ):
        q0, q1 = 512 * j, 512 * (j + 1)
        # --- projection for this q-tile: 5 M-chunks x 6 K matmuls ---
        for m in range(5):
            pt = psA.tile([128, 512], F32, tag="pa")
            for c in range(KC):
                nc.tensor.matmul(pt, wqkv[:, c, 128 * m:128 * (m + 1)],
                                 xT[:, c, q0:q1], start=(c == 0), stop=(c == KC - 1))
            dst, off = proj_dst(m)
            nc.vector.tensor_scalar(out=dst[:, off + q0:off + q1], in0=pt,
                                    scalar1=bqkv[:, m:m + 1], scalar2=None, op0=OP.add)
        # --- v transposes for blocks 4j..4j+3 -> fv chunks 4j+1..4j+4 ---
        for b in range(4 * j, 4 * (j + 1)):
            cidx = b + 1
            pt = psX.tile([128, 512], BF16, tag="ps", name="ptv")
            nc.tensor.transpose(pt[:, 0:128], v01T[:, 128 * b:128 * (b + 1)], ident_bf)
            nc.vector.tensor_copy(fv[0][:, cidx, 0:64], pt[:, 0:64])
            nc.vector.tensor_copy(fv[1][:, cidx, 0:64], pt[:, 64:128])
            pt2 = psX.tile([128, 512], BF16, tag="ps", name="ptv2")
            nc.tensor.transpose(pt2[:, 0:64], v2q2[0:64, 128 * b:128 * (b + 1)], ident_bf[0:64, 0:64])
            nc.vector.tensor_copy(fv[2][:, cidx, 0:64], pt2[:, 0:64])

        # --- attention ---
        nch = _nchunks(j)

        def s_slice(cidx):
            if cidx == 0:
                return 0, MT
            return MT + 128 * (cidx - 1), MT + 128 * cidx

        # per-head operand sources: (k-rows lo/hi, q-rows lo/hi, k tensor, q tensor)
        HSRC = ((0, 64, 0, 64, fkT01, qT01),
                (64, 128, 64, 128, fkT01, qT01),
                (64, 128, 64, 128, q2k2, v2q2))

        def emit_scores(h, cidx):
            flo, fhi, klo, khi, ksrc, qsrc = HSRC[h]
            slo, shi = s_slice(cidx)
            m = shi - slo
            st = psX.tile([128, 512], F32, tag="ps", name=f"st{h}")
            nc.tensor.matmul(st[0:m, :], ksrc[flo:fhi, slo:shi],
                             qsrc[klo:khi, q0:q1], start=True, stop=True)
            et = expp.tile([128, 512], BF16, tag="et", name=f"et{h}")
            k = cidx - 4 * j
            if cidx > 0 and k >= 1:
                qv = 128 * (k - 1)            # first valid local q col
                if qv > 0:
                    nc.gpsimd.memset(et[0:m, 0:qv], 0.0)
                nc.scalar.activation(et[0:m, qv:512], st[0:m, qv:512], AF.Exp)
                nc.gpsimd.tensor_tensor(out=et[0:m, qv:512], in0=et[0:m, qv:512],
                                        in1=m01[0:m, 0:512 - qv], op=OP.mult)
            else:
                nc.scalar.activation(et[0:m, :], st[0:m, :], AF.Exp)
            return et

        def emit_av(h, cidx, acc, et):
            slo, shi = s_slice(cidx)
            m = shi - slo
            nc.tensor.matmul(acc[0:65, :], fv[h][0:m, cidx, :], et[0:m, :],
                             start=(cidx == 0), stop=(cidx == nch - 1))

        def normalize(h, acc):
            dstg = smalls.tile([1, 512], F32, tag="dstg")
            nc.vector.tensor_copy(dstg, acc[64:65, :])
            den = smalls.tile([1, 512], F32, tag="den")
            nc.vector.reciprocal_approx_fast(den, dstg)
            bc = smalls.tile([64, 512], F32, tag="bc")
            nc.gpsimd.partition_broadcast(bc, den, channels=64)
            if dbg and h == 0 and j == 0:
                st0 = outst.tile([128, 512], F32, tag="ot")
                nc.vector.tensor_copy(st0[0:65, :], acc[0:65, :])
                nc.sync.dma_start(out=dbg["d_acc0"][:, :], in_=st0)
                nc.sync.dma_start(out=dbg["d_den0"][:, :], in_=den)
                nc.sync.dma_start(out=dbg["d_bc0"][:, :], in_=bc)
            if h == 0:
                dst = yA[0:64, q0:q1]
            elif h == 1:
                dst = yA[64:128, q0:q1]
            else:
                dst = yB[0:64, q0:q1]
            nc.vector.tensor_tensor(out=dst, in0=acc[0:64, :], in1=bc, op=OP.mult)

        accs = [psV.tile([128, 512], F32, tag="pv", name=f"acc{h}") for h in range(HPC)]
        for cidx in range(nch):
            ets = [emit_scores(h, cidx) for h in range(HPC)]
            for h in range(HPC):
                emit_av(h, cidx, accs[h], ets[h])
            if j > 0 and cidx == 1:
                # previous q-tile's output projection: PE work that overlaps
                # this tile's normalize chain instead of stalling on it
                emit_yproj(j - 1)
        for h in range(HPC):
            normalize(h, accs[h])
    emit_yproj(NJ - 1)

    if dbg:
        for name, src in (("d_xT", xT), ("d_qT01", qT01), ("d_fkT01", fkT01),
                          ("d_q2k2", q2k2), ("d_v01T", v01T), ("d_v2q2", v2q2),
                          ("d_fv0", fv[0]), ("d_fv2", fv[2]), ("d_yA", yA),
                          ("d_yB", yB), ("d_m01", m01), ("d_memT", memT)):
            flat = src
            if len(src.shape) == 3:
                flat = src.rearrange("p a b -> p (a b)")
            nc.sync.dma_start(out=dbg[name][:, :], in_=flat.bitcast(F32))


# ---------------- host side ----------------

_NC = None


def _get_nc():
    global _NC
    if _NC is None:
        _NC = build_nc()
    return _NC


def _shard_inputs(mem, x, Wqkv, bqkv, Wmem, bmem, Wproj, bproj):
    """Build the 8 per-core input maps."""
    f32 = np.float32
    mem, x = np.asarray(mem, f32), np.asarray(x, f32)
    Wqkv, bqkv = np.asarray(Wqkv, f32), np.asarray(bqkv, f32)
    Wmem, bmem = np.asarray(Wmem, f32), np.asarray(bmem, f32)
    Wproj, bproj = np.asarray(Wproj, f32), np.asarray(bproj, f32)

    in_maps = []
    for core in range(8):
        b, g = core // 4, core % 4
        hs = [HPC * g + i for i in range(HPC)]     # global head ids
        # q/k/v column slices in Wqkv: q block [0,C), k [C,2C), v [2C,3C); head h -> cols h*HD..
        def qc(h):
            return slice(HD * h, HD * (h + 1))
        def kc(h):
            return slice(C + HD * h, C + HD * (h + 1))
        def vc(h):
            return slice(2 * C + HD * h, 2 * C + HD * (h + 1))
        h0, h1, h2 = hs
        sc = np.float32(1.0 / np.sqrt(HD))         # fold score scale into q
        wq = np.concatenate([
            sc * Wqkv[:, qc(h0)], sc * Wqkv[:, qc(h1)],   # chunk0 [q0|q1]
            Wqkv[:, kc(h0)], Wqkv[:, kc(h1)],             # chunk1 [k0|k1]
            sc * Wqkv[:, qc(h2)], Wqkv[:, kc(h2)],        # chunk2 [q2|k2]
            Wqkv[:, vc(h0)], Wqkv[:, vc(h1)],             # chunk3 [v0|v1]
            Wqkv[:, vc(h2)], sc * Wqkv[:, qc(h2)],        # chunk4 [v2|q2dup]
        ], axis=1)
        bq = np.concatenate([
            sc * bqkv[qc(h0)], sc * bqkv[qc(h1)], bqkv[kc(h0)], bqkv[kc(h1)],
            sc * bqkv[qc(h2)], bqkv[kc(h2)], bqkv[vc(h0)], bqkv[vc(h1)],
            bqkv[vc(h2)], sc * bqkv[qc(h2)],
        ])[None, :]
        wm = np.concatenate([
            Wmem[:, kc(h0)], Wmem[:, kc(h1)],                    # k01 chunk
            np.zeros((C, HD), f32), Wmem[:, kc(h2)],             # [pad|k2]
            Wmem[:, vc(h0)], Wmem[:, vc(h1)], Wmem[:, vc(h2)],   # v3
        ], axis=1)
        bmk = np.concatenate([
            bmem[kc(h0)], bmem[kc(h1)], np.zeros(HD, f32), bmem[kc(h2)],
        ])[None, :]
        bmv = np.concatenate([bmem[vc(h0)], bmem[vc(h1)], bmem[vc(h2)]])[None, :]
        # wproj rows for these heads + bias row (bias only on g==0)
        wp = np.concatenate([
            Wproj[HD * h0:HD * (h0 + 1), :], Wproj[HD * h1:HD * (h1 + 1), :],
            Wproj[HD * h2:HD * (h2 + 1), :],
            (bproj[None, :] if g == 0 else np.zeros((1, C), f32)),
        ], axis=0)
        in_maps.append({
            "x": np.ascontiguousarray(x[b]),
            "mem": np.ascontiguousarray(mem[b]),
            "wqkv": np.ascontiguousarray(wq),
            "bqkv": np.ascontiguousarray(bq),
            "wmem": np.ascontiguousarray(wm),
            "bmemk": np.ascontiguousarray(bmk),
            "bmemv": np.ascontiguousarray(bmv),
            "wproj": np.ascontiguousarray(wp),
        })
    return in_maps


def run_on_hw(in_maps, trace=False):
    from concourse.bass_utils import run_bass_kernel_spmd
    nc = _get_nc()
    res = run_bass_kernel_spmd(nc, in_maps, core_ids=list(range(8)), trace=trace)
    return res


def kernel(mem, x, Wqkv, bqkv, Wmem, bmem, Wproj, bproj):
    in_maps = _shard_inputs(mem, x, Wqkv, bqkv, Wmem, bmem, Wproj, bproj)
    trace = bool(int(os.environ.get("KERNEL_TRACE", "0")))
    res = run_on_hw(in_maps, trace=trace)
    if trace:
        kernel.last_exec_time_ns = res.exec_time_ns
    out = np.zeros((B, T, C), np.float32)
    for core in range(8):
        out[core // 4] += res.results[core]["out"]
    return out
